# revision 6
# baseline (speedup 1.0000x reference)
"""Causal multi-head self-attention (B=4, S=2048, D=1024, H=16, RoPE) on 8
Trainium2 NeuronCores.

Sharding (hardcoded): core c handles batch b = c//2 and head group g = c%2
(8 of the 16 heads).  Data parallel over B, tensor parallel over heads for
the Wq/Wk/Wv projections and over Wo rows/columns: each core computes the
attention output for its 8 heads, the two cores of a pair AllGather their
(normalized) per-head-pair attention chunks, and each core then computes a
disjoint 512-wide column slice of the final Wo projection for its batch, so
the host only concatenates slices (no host-side arithmetic).

Kernel structure per head-pair (hp = 2 heads, 128 q/k/v dims):
  1. qT/kT/vT = W.T @ xT as [128 e, S] tiles straight from the tensor engine
     (x and weights are fed pre-transposed from the host, bf16).
  2. RoPE applied to qT/kT on the vector engine using host-built cos/sin
     tables.  Wq/Wk rows are host-permuted so each head's 64 dims are laid
     out [32 even-pair dims, 32 odd-pair dims], making the rotation two
     contiguous-block multiplies instead of a stride-2 shuffle.
  3. vT is transposed back to v [sk, d] blocks on the tensor engine and
     augmented with a ones column, so the PV matmul also produces the
     softmax denominator (row 64 of the PSUM output).
  4. Flash-style causal attention over [sk 128, sq 512] score blocks held
     transposed: QK^T -> exp (scalar engine, scale=1/8, no max subtraction
     -- scores are O(1) by construction) -> causal mask multiply (only on
     the 4 diagonal blocks) -> PV accumulation in PSUM.
  5. attnT chunk normalized (reciprocal of denominator row) -> AllGather
     with the paired core -> Wo partial matmul accumulated into SBUF.
"""

import numpy as np

D_MODEL = 1024
NUM_HEADS = 16
ROPE_THETA = 10000.0
DH = D_MODEL // NUM_HEADS  # 64
SQT = 512  # sq tile width (= PSUM bank width in f32)


# ---------------------------------------------------------------------------
# Device kernel builder
# ---------------------------------------------------------------------------

def build_kernel(n_cores: int = 8, S: int = 2048):
    import concourse.bass as bass
    import concourse.mybir as mybir
    import concourse.tile as tile
    from concourse import bacc

    F32 = mybir.dt.float32
    BF16 = mybir.dt.bfloat16
    Exp = mybir.ActivationFunctionType.Exp
    mult = mybir.AluOpType.mult
    add = mybir.AluOpType.add

    D = D_MODEL
    NC = D // 128          # 8 d-chunks
    NSB = S // 128         # s 128-blocks
    NSQ = S // SQT         # sq 512-tiles
    NHP = 4                # head pairs per core

    nc = bacc.Bacc("TRN2", target_bir_lowering=False, debug=False,
                   num_devices=n_cores)

    xT = nc.dram_tensor("xT", [128, NC, S], BF16, kind="ExternalInput")
    wqT = nc.dram_tensor("wqT", [128, NC, NHP, 128], BF16, kind="ExternalInput")
    wkT = nc.dram_tensor("wkT", [128, NC, NHP, 128], BF16, kind="ExternalInput")
    wvT = nc.dram_tensor("wvT", [128, NC, NHP, 128], BF16, kind="ExternalInput")
    woT = nc.dram_tensor("woT", [128, NC, SQT], BF16, kind="ExternalInput")
    cosT = nc.dram_tensor("cosT", [128, S], F32, kind="ExternalInput")
    sinT = nc.dram_tensor("sinT", [128, S], F32, kind="ExternalInput")
    masks = nc.dram_tensor("masks", [128, 4, SQT], BF16, kind="ExternalInput")
    out = nc.dram_tensor("out", [S, SQT], F32, kind="ExternalOutput")

    groups = [[2 * i, 2 * i + 1] for i in range(n_cores // 2)]

    with tile.TileContext(nc) as tc:
        with (
            tc.tile_pool(name="const", bufs=1) as constp,
            tc.tile_pool(name="w", bufs=2) as wp,
            tc.tile_pool(name="qk", bufs=2) as qkp,
            tc.tile_pool(name="v", bufs=2) as vp,
            tc.tile_pool(name="probs", bufs=6) as probsp,
            tc.tile_pool(name="rope", bufs=2) as ropep,
            tc.tile_pool(name="attn", bufs=2) as attnp,
            tc.tile_pool(name="ag", bufs=2) as agp,
            tc.tile_pool(name="acc", bufs=1) as accp,
            tc.tile_pool(name="small", bufs=3) as smallp,
            tc.tile_pool(name="norm", bufs=4) as normp,
            tc.tile_pool(name="psA", bufs=2, space="PSUM") as psA,
            tc.tile_pool(name="psQK", bufs=2, space="PSUM") as psQK,
            tc.tile_pool(name="psPV", bufs=2, space="PSUM") as psPV,
            tc.tile_pool(name="dram", bufs=2, space="DRAM") as dramp,
        ):
            # --- one-time loads -------------------------------------------
            xt_sb = constp.tile([128, NC, S], BF16, tag="xt")
            nc.sync.dma_start(xt_sb[:], xT[:])
            wo_sb = constp.tile([128, NC, SQT], BF16, tag="wo")
            nc.sync.dma_start(wo_sb[:], woT[:])
            cos_sb = constp.tile([128, S], F32, tag="cos")
            nc.sync.dma_start(cos_sb[:], cosT[:])
            sin_sb = constp.tile([128, S], F32, tag="sin")
            nc.sync.dma_start(sin_sb[:], sinT[:])
            mask_sb = constp.tile([128, 4, SQT], BF16, tag="mask")
            nc.sync.dma_start(mask_sb[:], masks[:])
            ident = constp.tile([128, 128], BF16, tag="ident")
            from concourse.masks import make_identity
            make_identity(nc, ident[:])

            out_acc = accp.tile([128, NSB, SQT], F32, tag="oacc")

            for hp in range(NHP):
                # --- load this head-pair's weight slices ------------------
                wq_sb = wp.tile([128, NC, 128], BF16, tag="wq")
                nc.sync.dma_start(wq_sb[:], wqT[:, :, hp, :])
                wk_sb = wp.tile([128, NC, 128], BF16, tag="wk")
                nc.sync.dma_start(wk_sb[:], wkT[:, :, hp, :])
                wv_sb = wp.tile([128, NC, 128], BF16, tag="wv")
                nc.sync.dma_start(wv_sb[:], wvT[:, :, hp, :])

                # --- q/k projections + RoPE -------------------------------
                qT2 = qkp.tile([128, S], BF16, tag="qT")
                kT2 = qkp.tile([128, S], BF16, tag="kT")
                for w_sb, dst in ((wq_sb, qT2), (wk_sb, kT2)):
                    for j in range(NSQ):
                        jsl = bass.ts(j, SQT)
                        ps = psA.tile([128, SQT], F32, tag="psA")
                        for c in range(NC):
                            nc.tensor.matmul(
                                ps[:], w_sb[:, c, :], xt_sb[:, c, jsl],
                                start=(c == 0), stop=(c == NC - 1))
                        # rope: dst = ps*cos + swap32(ps)*sin_signed
                        t1 = ropep.tile([128, SQT], F32, tag="t1")
                        nc.vector.tensor_tensor(
                            out=t1[:], in0=ps[:], in1=cos_sb[:, jsl], op=mult)
                        t2 = ropep.tile([128, SQT], F32, tag="t2")
                        for blk in range(4):
                            sw = 32 * (blk ^ 1)
                            nc.vector.tensor_tensor(
                                out=t2[32 * blk:32 * blk + 32, :],
                                in0=ps[sw:sw + 32, :],
                                in1=sin_sb[32 * blk:32 * blk + 32, jsl],
                                op=mult)
                        nc.vector.tensor_tensor(
                            out=dst[:, jsl], in0=t1[:], in1=t2[:], op=add)

                # --- v projection + transpose + ones column ---------------
                vaug = vp.tile([128, NSB, 130], BF16, tag="vaug")
                nc.gpsimd.memset(vaug[:, :, 64], 1.0)
                nc.gpsimd.memset(vaug[:, :, 129], 1.0)
                for j in range(NSQ):
                    jsl = bass.ts(j, SQT)
                    ps = psA.tile([128, SQT], F32, tag="psA")
                    for c in range(NC):
                        nc.tensor.matmul(
                            ps[:], wv_sb[:, c, :], xt_sb[:, c, jsl],
                            start=(c == 0), stop=(c == NC - 1))
                    vt_sb = smallp.tile([128, SQT], BF16, tag="vt")
                    nc.vector.tensor_copy(vt_sb[:], ps[:])
                    for t in range(SQT // 128):
                        sb = j * (SQT // 128) + t
                        tp = psA.tile([128, 128], BF16, tag="psA")
                        nc.tensor.transpose(
                            tp[:], vt_sb[:, bass.ts(t, 128)], ident[:])
                        nc.vector.tensor_copy(vaug[:, sb, 0:64], tp[:, 0:64])
                        nc.vector.tensor_copy(vaug[:, sb, 65:129], tp[:, 64:128])

                # --- causal attention, scores held transposed -------------
                attnT2 = attnp.tile([128, S], BF16, tag="attnT")
                for h in range(2):
                    hsl = slice(64 * h, 64 * h + 64)
                    for j in range(NSQ):
                        jsl = bass.ts(j, SQT)
                        pv = psPV.tile([128, SQT], F32, tag="pv")
                        n_sk = (SQT // 128) * j + 4
                        for i in range(n_sk):
                            qk = psQK.tile([128, SQT], F32, tag="qk")
                            nc.tensor.matmul(
                                qk[:], kT2[hsl, bass.ts(i, 128)],
                                qT2[hsl, jsl], start=True, stop=True)
                            pr = probsp.tile([128, SQT], BF16, tag="probs")
                            nc.scalar.activation(pr[:], qk[:], Exp, scale=0.125)
                            m = i - (SQT // 128) * j
                            if m >= 0:
                                nc.vector.tensor_tensor(
                                    out=pr[:], in0=pr[:], in1=mask_sb[:, m, :],
                                    op=mult)
                            nc.tensor.matmul(
                                pv[0:65, :], vaug[:, i, 65 * h:65 * h + 65],
                                pr[:], start=(i == 0), stop=(i == n_sk - 1))
                        rec = normp.tile([1, SQT], F32, tag="rec")
                        nc.vector.reciprocal(rec[:], pv[64:65, :])
                        rec64 = normp.tile([64, SQT], F32, tag="rec64")
                        nc.gpsimd.partition_broadcast(rec64[:], rec[:])
                        nc.vector.tensor_tensor(
                            out=attnT2[hsl, jsl], in0=pv[0:64, :],
                            in1=rec64[:], op=mult)

                # --- exchange attention chunks with paired core -----------
                ag_in = dramp.tile([128, S], BF16, tag="ag_in")
                nc.sync.dma_start(ag_in[:], attnT2[:])
                ag_out = dramp.tile([2, 128, S], BF16, tag="ag_out")
                nc.gpsimd.collective_compute(
                    "AllGather", mybir.AluOpType.bypass,
                    ins=[ag_in[:].opt()], outs=[ag_out[:].opt()],
                    replica_groups=groups)
                ag0 = agp.tile([128, S], BF16, tag="ag0")
                nc.sync.dma_start(ag0[:], ag_out[0])
                ag1 = agp.tile([128, S], BF16, tag="ag1")
                nc.sync.dma_start(ag1[:], ag_out[1])

                # --- Wo partial for this head pair ------------------------
                for sb in range(NSB):
                    ssl = bass.ts(sb, 128)
                    ps = psA.tile([128, SQT], F32, tag="psA")
                    nc.tensor.matmul(ps[:], ag0[:, ssl], wo_sb[:, hp, :],
                                     start=True, stop=False)
                    nc.tensor.matmul(ps[:], ag1[:, ssl], wo_sb[:, NC // 2 + hp, :],
                                     start=False, stop=True)
                    if hp == 0:
                        nc.vector.tensor_copy(out_acc[:, sb, :], ps[:])
                    else:
                        nc.vector.tensor_tensor(
                            out=out_acc[:, sb, :], in0=out_acc[:, sb, :],
                            in1=ps[:], op=add)

            # --- write the output slice -----------------------------------
            nc.sync.dma_start(
                out.rearrange("(n p) e -> p n e", p=128), out_acc[:])

    nc.compile()
    return nc


# ---------------------------------------------------------------------------
# Host-side sharding / unsharding
# ---------------------------------------------------------------------------

def _host_inputs(x, Wq, Wk, Wv, Wo, token_positions, n_cores, S):
    import ml_dtypes
    bf16 = ml_dtypes.bfloat16
    D = D_MODEL
    NC = D // 128
    NHP = 4

    # rope tables (period-32 partition layout, sin carries the block sign)
    pos = np.asarray(token_positions).astype(np.float32)  # (S,)
    i32 = np.arange(32, dtype=np.float32)
    inv_freq = ROPE_THETA ** (-i32 / 32.0)
    ang = pos[None, :] * inv_freq[:, None]              # (32, S)
    cos32, sin32 = np.cos(ang), np.sin(ang)
    cosT = np.tile(cos32, (4, 1)).astype(np.float32)    # (128, S)
    sinT = np.concatenate([-sin32, sin32, -sin32, sin32]).astype(np.float32)

    # causal mask patterns for the 4 diagonal block offsets
    p = np.arange(128)[:, None]
    f = np.arange(SQT)[None, :]
    masks = np.stack([(p + 128 * m <= f) for m in range(4)], axis=1)
    masks = masks.astype(bf16)                          # (128, 4, 512)

    # de-interleaving row permutation for q/k (per head: evens then odds)
    def qk_rows(g):
        rows = []
        for h in range(8 * g, 8 * g + 8):
            rows += [h * DH + 2 * i for i in range(DH // 2)]
            rows += [h * DH + 2 * i + 1 for i in range(DH // 2)]
        return rows

    def wqk_layout(W, g):
        # (D, 512) -> [128, NC, NHP, 128]
        t = W[qk_rows(g), :].T.astype(bf16)
        return np.ascontiguousarray(
            t.reshape(NC, 128, NHP, 128).transpose(1, 0, 2, 3))

    def wv_layout(W, g):
        t = W[512 * g:512 * g + 512, :].T.astype(bf16)
        return np.ascontiguousarray(
            t.reshape(NC, 128, NHP, 128).transpose(1, 0, 2, 3))

    def wo_layout(W, g):
        t = W.T[:, 512 * g:512 * g + 512].astype(bf16)  # (D, 512)
        return np.ascontiguousarray(t.reshape(NC, 128, SQT).transpose(1, 0, 2))

    in_maps = []
    for c in range(n_cores):
        b, g = c // 2, c % 2
        xb = np.ascontiguousarray(x[b].T).astype(bf16)  # (D, S)
        in_maps.append({
            "xT": np.ascontiguousarray(
                xb.reshape(NC, 128, S).transpose(1, 0, 2)),
            "wqT": wqk_layout(Wq, g),
            "wkT": wqk_layout(Wk, g),
            "wvT": wv_layout(Wv, g),
            "woT": wo_layout(Wo, g),
            "cosT": cosT,
            "sinT": sinT,
            "masks": masks,
        })
    return in_maps


def _assemble(results, n_cores, S):
    B = n_cores // 2
    full = np.empty((B, S, D_MODEL), dtype=np.float32)
    for c in range(n_cores):
        b, g = c // 2, c % 2
        full[b, :, 512 * g:512 * g + 512] = results[c]["out"]
    return full


# ---------------------------------------------------------------------------
# Entry point
# ---------------------------------------------------------------------------

_NC_CACHE = {}


def _get_nc(n_cores, S):
    key = (n_cores, S)
    if key not in _NC_CACHE:
        _NC_CACHE[key] = build_kernel(n_cores, S)
    return _NC_CACHE[key]


def kernel(x, Wq, Wk, Wv, Wo, token_positions, _trace=False, _tmpdir=None):
    from concourse.bass_utils import run_bass_kernel_spmd

    x = np.asarray(x)
    B, S, D = x.shape
    n_cores = 2 * B
    nc = _get_nc(n_cores, S)
    in_maps = _host_inputs(np.asarray(x), np.asarray(Wq), np.asarray(Wk),
                           np.asarray(Wv), np.asarray(Wo),
                           np.asarray(token_positions), n_cores, S)
    res = run_bass_kernel_spmd(nc, in_maps, core_ids=list(range(n_cores)),
                               trace=_trace, tmpdir=_tmpdir)
    out = _assemble(res.results, n_cores, S)
    if _trace:
        return out, res
    return out


# revision 24
# speedup vs baseline: 1.6268x; 1.6268x over previous
"""Causal multi-head self-attention (B=4, S=2048, D=1024, H=16, RoPE) on 8
Trainium2 NeuronCores.

Sharding (hardcoded): core c handles batch b = c//2 and head group g = c%2
(8 of the 16 heads).  Data parallel over B, tensor parallel over heads for
the Wq/Wk/Wv projections and over Wo rows/columns: each core computes the
attention output for its 8 heads, the two cores of a pair AllGather their
(normalized) per-head-pair attention chunks, and each core then computes a
disjoint 512-wide column slice of the final Wo projection for its batch, so
the host only concatenates slices (no host-side arithmetic).

Kernel structure per head-pair (hp = 2 heads, 128 q/k/v dims):
  1. qT/kT/vT = W.T @ xT as [128 e, S] tiles straight from the tensor engine
     (x and weights are fed pre-transposed from the host, bf16).
  2. RoPE applied to qT/kT on the vector engine using host-built cos/sin
     tables.  Wq/Wk rows are host-permuted so each head's 64 dims are laid
     out [32 even-pair dims, 32 odd-pair dims], making the rotation two
     contiguous-block multiplies instead of a stride-2 shuffle.
  3. vT is transposed back to v [sk, d] blocks on the tensor engine and
     augmented with a ones column, so the PV matmul also produces the
     softmax denominator (row 64 of the PSUM output).
  4. Flash-style causal attention over [sk 128, sq 512] score blocks held
     transposed: QK^T -> exp (scalar engine, scale=1/8, no max subtraction
     -- scores are O(1) by construction) -> causal mask multiply (only on
     the 4 diagonal blocks) -> PV accumulation in PSUM.
  5. attnT chunk normalized (reciprocal of denominator row) -> AllGather
     with the paired core -> Wo partial matmul accumulated into SBUF.
"""

import numpy as np

D_MODEL = 1024
NUM_HEADS = 16
ROPE_THETA = 10000.0
DH = D_MODEL // NUM_HEADS  # 64
SQT = 512  # sq tile width (= PSUM bank width in f32)


# ---------------------------------------------------------------------------
# Device kernel builder
# ---------------------------------------------------------------------------

def build_kernel(n_cores: int = 8, S: int = 2048):
    import concourse.bass as bass
    import concourse.mybir as mybir
    import concourse.tile as tile
    from concourse import bacc

    F32 = mybir.dt.float32
    BF16 = mybir.dt.bfloat16
    Exp = mybir.ActivationFunctionType.Exp
    Recip = mybir.ActivationFunctionType.Reciprocal
    mult = mybir.AluOpType.mult
    add = mybir.AluOpType.add

    D = D_MODEL
    NC = D // 128          # 8 d-chunks
    NSB = S // 128         # s 128-blocks
    NSQ = S // SQT         # sq 512-tiles
    NHP = 4                # head pairs per core

    nc = bacc.Bacc("TRN2", target_bir_lowering=False, debug=False,
                   num_devices=n_cores)

    xT = nc.dram_tensor("xT", [128, NC, S], BF16, kind="ExternalInput")
    wqT = nc.dram_tensor("wqT", [128, NC, NHP, 128], BF16, kind="ExternalInput")
    wkT = nc.dram_tensor("wkT", [128, NC, NHP, 128], BF16, kind="ExternalInput")
    wvT = nc.dram_tensor("wvT", [128, NC, NHP, 128], BF16, kind="ExternalInput")
    woT = nc.dram_tensor("woT", [128, NC, SQT], BF16, kind="ExternalInput")
    cosT = nc.dram_tensor("cosT", [128, S], F32, kind="ExternalInput")
    sinT = nc.dram_tensor("sinT", [128, S], F32, kind="ExternalInput")
    masks = nc.dram_tensor("masks", [128, 4, SQT], BF16, kind="ExternalInput")
    out = nc.dram_tensor("out", [S, SQT], F32, kind="ExternalOutput")

    groups = [[2 * i, 2 * i + 1] for i in range(n_cores // 2)]

    with tile.TileContext(nc) as tc:
        with (
            tc.tile_pool(name="const", bufs=1) as constp,
            tc.tile_pool(name="w", bufs=2) as wp,
            tc.tile_pool(name="qk", bufs=2) as qkp,
            tc.tile_pool(name="v", bufs=2) as vp,
            tc.tile_pool(name="probs", bufs=6) as probsp,
            tc.tile_pool(name="rope", bufs=2) as ropep,
            tc.tile_pool(name="attn", bufs=2) as attnp,
            tc.tile_pool(name="ag", bufs=2) as agp,
            tc.tile_pool(name="acc", bufs=1) as accp,
            tc.tile_pool(name="small", bufs=3) as smallp,
            tc.tile_pool(name="norm", bufs=2) as normp,
            tc.tile_pool(name="unn", bufs=1) as unnp,
            tc.tile_pool(name="psA", bufs=2, space="PSUM") as psA,
            tc.tile_pool(name="psQK", bufs=2, space="PSUM") as psQK,
            tc.tile_pool(name="psPV", bufs=2, space="PSUM") as psPV,
            tc.tile_pool(name="dram", bufs=2, space="DRAM") as dramp,
        ):
            # --- one-time loads -------------------------------------------
            xt_sb = constp.tile([128, NC, S], BF16, tag="xt")
            for c in range(NC):
                nc.sync.dma_start(xt_sb[:, c, :], xT[:, c, :])
            wo_sb = constp.tile([128, NC, SQT], BF16, tag="wo")
            nc.sync.dma_start(wo_sb[:], woT[:])
            cos_sb = constp.tile([128, S], F32, tag="cos")
            nc.sync.dma_start(cos_sb[:], cosT[:])
            sin_sb = constp.tile([128, S], F32, tag="sin")
            nc.sync.dma_start(sin_sb[:], sinT[:])
            mask_sb = constp.tile([128, 4, SQT], BF16, tag="mask")
            nc.sync.dma_start(mask_sb[:], masks[:])
            ident = constp.tile([128, 128], BF16, tag="ident")
            from concourse.masks import make_identity
            make_identity(nc, ident[:])

            out_acc = accp.tile([128, NSB, SQT], F32, tag="oacc")

            def emit_wo_block(hp, ag0, ag1, sb, final):
                ssl = bass.ts(sb, 128)
                ps = psA.tile([128, SQT], F32, tag="psA")
                nc.tensor.matmul(ps[:], ag0[:, ssl], wo_sb[:, hp, :],
                                 start=True, stop=False)
                nc.tensor.matmul(ps[:], ag1[:, ssl],
                                 wo_sb[:, NC // 2 + hp, :],
                                 start=False, stop=True)
                if hp == 0:
                    nc.vector.tensor_copy(out_acc[:, sb, :], ps[:])
                else:
                    nc.vector.tensor_tensor(
                        out=out_acc[:, sb, :], in0=out_acc[:, sb, :],
                        in1=ps[:], op=add)
                if final:
                    nc.sync.dma_start(out[ssl, :], out_acc[:, sb, :])

            prev_wo = None
            for hp in range(NHP):
                # --- load this head-pair's weight slices ------------------
                wq_sb = wp.tile([128, NC, 128], BF16, tag="wq")
                nc.sync.dma_start(wq_sb[:], wqT[:, :, hp, :])
                wk_sb = wp.tile([128, NC, 128], BF16, tag="wk")
                nc.sync.dma_start(wk_sb[:], wkT[:, :, hp, :])
                wv_sb = wp.tile([128, NC, 128], BF16, tag="wv")
                nc.sync.dma_start(wv_sb[:], wvT[:, :, hp, :])

                # --- q/k projections + RoPE -------------------------------
                qT2 = qkp.tile([128, S], BF16, tag="qT")
                kT2 = qkp.tile([128, S], BF16, tag="kT")
                SWAP16 = list(range(16, 32)) + list(range(16))
                for w_sb, dst in ((wq_sb, qT2), (wk_sb, kT2)):
                    for j in range(NSQ):
                        jsl = bass.ts(j, SQT)
                        ps = psA.tile([128, SQT], F32, tag="psA")
                        for c in range(NC):
                            nc.tensor.matmul(
                                ps[:], w_sb[:, c, :], xt_sb[:, c, jsl],
                                start=(c == 0), stop=(c == NC - 1))
                        # rope: dst = ps*cos + shuffle16(ps)*sin_signed
                        # (host lays pair partners 16 apart in each 32-group)
                        t1 = ropep.tile([128, SQT], F32, tag="t1")
                        nc.vector.tensor_tensor(
                            out=t1[:], in0=ps[:], in1=cos_sb[:, jsl], op=mult)
                        sh = ropep.tile([128, SQT], F32, tag="sh")
                        nc.vector.stream_shuffle(sh[:], ps[:], SWAP16)
                        t2 = ropep.tile([128, SQT], F32, tag="t2")
                        nc.vector.tensor_tensor(
                            out=t2[:], in0=sh[:], in1=sin_sb[:, jsl], op=mult)
                        nc.vector.tensor_tensor(
                            out=dst[:, jsl], in0=t1[:], in1=t2[:], op=add)

                # --- v projection + transpose + ones column ---------------
                vaug = vp.tile([128, NSB, 130], BF16, tag="vaug")
                nc.gpsimd.memset(vaug[:, :, 64], 1.0)
                nc.gpsimd.memset(vaug[:, :, 129], 1.0)
                for j in range(NSQ):
                    jsl = bass.ts(j, SQT)
                    ps = psA.tile([128, SQT], F32, tag="psA")
                    for c in range(NC):
                        nc.tensor.matmul(
                            ps[:], wv_sb[:, c, :], xt_sb[:, c, jsl],
                            start=(c == 0), stop=(c == NC - 1))
                    vt_sb = smallp.tile([128, SQT], BF16, tag="vt")
                    nc.vector.tensor_copy(vt_sb[:], ps[:])
                    for t in range(SQT // 128):
                        sb = j * (SQT // 128) + t
                        tp = psA.tile([128, 128], BF16, tag="psA")
                        nc.tensor.transpose(
                            tp[:], vt_sb[:, bass.ts(t, 128)], ident[:])
                        nc.vector.tensor_copy(vaug[:, sb, 0:64], tp[:, 0:64])
                        nc.vector.tensor_copy(vaug[:, sb, 65:129], tp[:, 64:128])

                # --- causal attention, scores held transposed -------------
                # Both heads share each [sk 128, sq 512] step: their score
                # blocks land in adjacent PSUM banks so exp and mask run as
                # single [128, 1024] ops.  PV outputs are staged to SBUF
                # unnormalized; denominator rows stack at 32-aligned
                # partitions so one reciprocal covers four of them.
                attnT2 = attnp.tile([128, S], BF16, tag="attnT")
                unnorm = unnp.tile([128, S], F32, tag="unnorm")
                den = []
                for h in range(2):
                    den_t = normp.tile([128, SQT], F32, tag="den")
                    nc.gpsimd.memset(den_t[:], 1.0)
                    den.append(den_t)
                for j in range(NSQ):
                    jsl = bass.ts(j, SQT)
                    pv0 = psPV.tile([128, SQT], F32, tag="pv")
                    pv1 = psPV.tile([128, SQT], F32, tag="pv")
                    n_sk = (SQT // 128) * j + 4
                    for i in range(n_sk):
                        qk2 = psQK.tile([128, 2 * SQT], F32, tag="qk")
                        for h in range(2):
                            nc.tensor.matmul(
                                qk2[:, bass.ts(h, SQT)],
                                kT2[64 * h:64 * h + 64, bass.ts(i, 128)],
                                qT2[64 * h:64 * h + 64, jsl],
                                start=True, stop=True)
                        pr2 = probsp.tile([128, 2 * SQT], BF16, tag="probs")
                        nc.scalar.activation(pr2[:], qk2[:], Exp, scale=0.125)
                        m = i - (SQT // 128) * j
                        if m >= 0:
                            nc.vector.tensor_tensor(
                                out=pr2[:].rearrange("p (two f) -> p two f", two=2),
                                in0=pr2[:].rearrange("p (two f) -> p two f", two=2),
                                in1=mask_sb[:, m, :].unsqueeze(1).broadcast_to(
                                    [128, 2, SQT]),
                                op=mult)
                        for h, pv in ((0, pv0), (1, pv1)):
                            nc.tensor.matmul(
                                pv[0:65, :], vaug[:, i, 65 * h:65 * h + 65],
                                pr2[:, bass.ts(h, SQT)],
                                start=(i == 0), stop=(i == n_sk - 1))
                    for h, pv in ((0, pv0), (1, pv1)):
                        nc.vector.tensor_copy(
                            unnorm[64 * h:64 * h + 64, jsl], pv[0:64, :])
                        nc.vector.tensor_copy(
                            den[h][32 * j:32 * j + 1, :], pv[64:65, :])
                    # Wo of the previous head pair, interleaved as tensor-
                    # engine filler while exp gates this pair's attention
                    if prev_wo is not None:
                        for sb in range(4 * j, 4 * j + 4):
                            emit_wo_block(*prev_wo, sb, final=False)
                for h in range(2):
                    rec = normp.tile([128, SQT], F32, tag="recb")
                    nc.vector.reciprocal(rec[:], den[h][:])
                    for j in range(NSQ):
                        jsl = bass.ts(j, SQT)
                        # partition_broadcast honors no AP offsets on HW:
                        # stage the row to a base-0 [1,512] tile, broadcast
                        # full-tile, then slice at in0's base partition.
                        r1 = normp.tile([1, SQT], F32, tag="r1")
                        nc.vector.tensor_copy(r1[:], rec[32 * j:32 * j + 1, :])
                        rec128 = normp.tile([128, SQT], F32, tag="rec128")
                        nc.gpsimd.partition_broadcast(rec128[:], r1[:])
                        nc.vector.tensor_tensor(
                            out=attnT2[64 * h:64 * h + 64, jsl],
                            in0=unnorm[64 * h:64 * h + 64, jsl],
                            in1=rec128[64 * h:64 * h + 64, :], op=mult)

                # --- exchange attention chunks with paired core -----------
                ag_in = dramp.tile([128, S], BF16, tag="ag_in")
                nc.sync.dma_start(ag_in[:], attnT2[:])
                ag_out = dramp.tile([2, 128, S], BF16, tag="ag_out")
                nc.gpsimd.collective_compute(
                    "AllGather", mybir.AluOpType.bypass,
                    ins=[ag_in[:].opt()], outs=[ag_out[:].opt()],
                    replica_groups=groups)
                ag0 = agp.tile([128, S], BF16, tag="ag0")
                nc.sync.dma_start(ag0[:], ag_out[0])
                ag1 = agp.tile([128, S], BF16, tag="ag1")
                nc.sync.dma_start(ag1[:], ag_out[1])
                prev_wo = (hp, ag0, ag1)

            for sb in range(NSB):
                emit_wo_block(*prev_wo, sb, final=True)

    nc.compile()
    return nc


# ---------------------------------------------------------------------------
# Host-side sharding / unsharding
# ---------------------------------------------------------------------------

def _host_inputs(x, Wq, Wk, Wv, Wo, token_positions, n_cores, S):
    import ml_dtypes
    bf16 = ml_dtypes.bfloat16
    D = D_MODEL
    NC = D // 128
    NHP = 4

    # rope tables.  Partition layout within each head (64 partitions):
    # [e0..e15, o0..o15, e16..e31, o16..o31] -- the rotation partner sits
    # 16 partitions away inside the same 32-group, so the kernel's
    # stream_shuffle (a per-32-group lane shuffle) can realize the swap.
    pos = np.asarray(token_positions).astype(np.float32)  # (S,)
    i32 = np.arange(32, dtype=np.float32)
    inv_freq = ROPE_THETA ** (-i32 / 32.0)
    ang = pos[None, :] * inv_freq[:, None]              # (32, S)
    p = np.arange(128)
    pp = p % 64
    g, o = pp // 32, pp % 32
    freq_idx = 16 * g + (o % 16)                        # (128,)
    sign = np.where(o % 32 < 16, -1.0, 1.0)             # even slots: -sin
    cosT = np.cos(ang[freq_idx, :]).astype(np.float32)  # (128, S)
    sinT = (sign[:, None] * np.sin(ang[freq_idx, :])).astype(np.float32)

    # causal mask patterns for the 4 diagonal block offsets
    p = np.arange(128)[:, None]
    f = np.arange(SQT)[None, :]
    masks = np.stack([(p + 128 * m <= f) for m in range(4)], axis=1)
    masks = masks.astype(bf16)                          # (128, 4, 512)

    # de-interleaving row permutation for q/k (per head: evens then odds)
    def qk_rows(g):
        rows = []
        for h in range(8 * g, 8 * g + 8):
            rows += [h * DH + 2 * i for i in range(16)]
            rows += [h * DH + 2 * i + 1 for i in range(16)]
            rows += [h * DH + 2 * i for i in range(16, 32)]
            rows += [h * DH + 2 * i + 1 for i in range(16, 32)]
        return rows

    def wqk_layout(W, g):
        # (D, 512) -> [128, NC, NHP, 128]
        t = W[qk_rows(g), :].T.astype(bf16)
        return np.ascontiguousarray(
            t.reshape(NC, 128, NHP, 128).transpose(1, 0, 2, 3))

    def wv_layout(W, g):
        t = W[512 * g:512 * g + 512, :].T.astype(bf16)
        return np.ascontiguousarray(
            t.reshape(NC, 128, NHP, 128).transpose(1, 0, 2, 3))

    def wo_layout(W, g):
        t = W.T[:, 512 * g:512 * g + 512].astype(bf16)  # (D, 512)
        return np.ascontiguousarray(t.reshape(NC, 128, SQT).transpose(1, 0, 2))

    in_maps = []
    for c in range(n_cores):
        b, g = c // 2, c % 2
        xb = np.ascontiguousarray(x[b].T).astype(bf16)  # (D, S)
        in_maps.append({
            "xT": np.ascontiguousarray(
                xb.reshape(NC, 128, S).transpose(1, 0, 2)),
            "wqT": wqk_layout(Wq, g),
            "wkT": wqk_layout(Wk, g),
            "wvT": wv_layout(Wv, g),
            "woT": wo_layout(Wo, g),
            "cosT": cosT,
            "sinT": sinT,
            "masks": masks,
        })
    return in_maps


def _assemble(results, n_cores, S):
    B = n_cores // 2
    full = np.empty((B, S, D_MODEL), dtype=np.float32)
    for c in range(n_cores):
        b, g = c // 2, c % 2
        full[b, :, 512 * g:512 * g + 512] = results[c]["out"]
    return full


# ---------------------------------------------------------------------------
# Entry point
# ---------------------------------------------------------------------------

_NC_CACHE = {}


def _get_nc(n_cores, S):
    key = (n_cores, S)
    if key not in _NC_CACHE:
        _NC_CACHE[key] = build_kernel(n_cores, S)
    return _NC_CACHE[key]


def kernel(x, Wq, Wk, Wv, Wo, token_positions, _trace=False, _tmpdir=None):
    from concourse.bass_utils import run_bass_kernel_spmd

    x = np.asarray(x)
    B, S, D = x.shape
    n_cores = 2 * B
    nc = _get_nc(n_cores, S)
    in_maps = _host_inputs(np.asarray(x), np.asarray(Wq), np.asarray(Wk),
                           np.asarray(Wv), np.asarray(Wo),
                           np.asarray(token_positions), n_cores, S)
    res = run_bass_kernel_spmd(nc, in_maps, core_ids=list(range(n_cores)),
                               trace=_trace, tmpdir=_tmpdir)
    out = _assemble(res.results, n_cores, S)
    if _trace:
        return out, res
    return out


# revision 25
# speedup vs baseline: 1.6344x; 1.0046x over previous
"""Causal multi-head self-attention (B=4, S=2048, D=1024, H=16, RoPE) on 8
Trainium2 NeuronCores.

Sharding (hardcoded): core c handles batch b = c//2 and head group g = c%2
(8 of the 16 heads).  Data parallel over B, tensor parallel over heads for
the Wq/Wk/Wv projections and over Wo rows/columns: each core computes the
attention output for its 8 heads, the two cores of a pair AllGather their
(normalized) per-head-pair attention chunks in 512-column slices, and each
core then computes a disjoint 512-wide column slice of the final Wo
projection for its batch, so the host only concatenates slices (no
host-side arithmetic).

Compute is bf16 on the tensor engine (f32 PSUM accumulation) throughout.

Per head pair (hp = 2 heads = 128 q/k/v dims):
  1. qT/kT/vT = W.T @ xT as [128 e, S] tiles straight off the tensor engine
     (x and weights fed pre-transposed from the host).
  2. RoPE on the vector engine: the PSUM result is evacuated to SBUF bf16
     first (frees the PSUM slot after one pass), then rotated with
     host-built cos/sin tables.  Wq/Wk rows are host-permuted so each
     rotation partner sits 16 partitions away within a 32-partition group,
     which a single stream_shuffle realizes.
  3. vT is transposed back to v [sk, d] blocks on the tensor engine and
     augmented with a ones column so the PV matmul also produces the
     softmax denominator (PSUM row 64).
  4. Flash-style causal attention over [sk 128, sq 512] score blocks held
     transposed; the two heads' blocks land in adjacent PSUM banks so exp
     (scalar engine, scale 1/8, no max subtraction -- scores are O(1) by
     construction) and the causal mask multiply run as [128, 1024] ops.
  5. Per sq-slice: PV outputs staged unnormalized to SBUF, denominators
     stacked at partitions {0,32} of a staging tile so one reciprocal
     serves both heads, normalization via gpsimd partition-broadcast
     (full-tile only -- HW ignores AP offsets), then a per-slice AllGather
     with the paired core.  Wo partials for gathered slices are emitted
     interleaved into later attention steps as tensor-engine filler.
"""

import numpy as np

D_MODEL = 1024
NUM_HEADS = 16
ROPE_THETA = 10000.0
DH = D_MODEL // NUM_HEADS  # 64
SQT = 512  # sq tile width (= PSUM bank width in f32)


# ---------------------------------------------------------------------------
# Device kernel builder
# ---------------------------------------------------------------------------

def build_kernel(n_cores: int = 8, S: int = 2048):
    import concourse.bass as bass
    import concourse.mybir as mybir
    import concourse.tile as tile
    from concourse import bacc
    from concourse.masks import make_identity

    F32 = mybir.dt.float32
    BF16 = mybir.dt.bfloat16
    Exp = mybir.ActivationFunctionType.Exp
    mult = mybir.AluOpType.mult
    add = mybir.AluOpType.add

    D = D_MODEL
    NC = D // 128          # 8 d-chunks
    NSB = S // 128         # s 128-blocks
    NSQ = S // SQT         # sq 512-tiles
    NHP = 4                # head pairs per core
    SWAP16 = list(range(16, 32)) + list(range(16))

    nc = bacc.Bacc("TRN2", target_bir_lowering=False, debug=False,
                   num_devices=n_cores)

    xT = nc.dram_tensor("xT", [128, NC, S], BF16, kind="ExternalInput")
    wqT = nc.dram_tensor("wqT", [128, NC, NHP, 128], BF16, kind="ExternalInput")
    wkT = nc.dram_tensor("wkT", [128, NC, NHP, 128], BF16, kind="ExternalInput")
    wvT = nc.dram_tensor("wvT", [128, NC, NHP, 128], BF16, kind="ExternalInput")
    woT = nc.dram_tensor("woT", [128, NC, SQT], BF16, kind="ExternalInput")
    cosT = nc.dram_tensor("cosT", [128, S], BF16, kind="ExternalInput")
    sinT = nc.dram_tensor("sinT", [128, S], BF16, kind="ExternalInput")
    masks = nc.dram_tensor("masks", [128, 4, SQT], BF16, kind="ExternalInput")
    out = nc.dram_tensor("out", [S, SQT], F32, kind="ExternalOutput")

    groups = [[2 * i, 2 * i + 1] for i in range(n_cores // 2)]

    with tile.TileContext(nc) as tc:
        with (
            tc.tile_pool(name="const", bufs=1) as constp,
            tc.tile_pool(name="w", bufs=2) as wp,
            tc.tile_pool(name="qk", bufs=2) as qkp,
            tc.tile_pool(name="v", bufs=2) as vp,
            tc.tile_pool(name="probs", bufs=6) as probsp,
            tc.tile_pool(name="rope", bufs=2) as ropep,
            tc.tile_pool(name="attn", bufs=2) as attnp,
            tc.tile_pool(name="ag", bufs=4) as agp,
            tc.tile_pool(name="acc", bufs=1) as accp,
            tc.tile_pool(name="small", bufs=3) as smallp,
            tc.tile_pool(name="norm", bufs=3) as normp,
            tc.tile_pool(name="unn", bufs=1) as unnp,
            tc.tile_pool(name="psA", bufs=2, space="PSUM") as psA,
            tc.tile_pool(name="psQK", bufs=2, space="PSUM") as psQK,
            tc.tile_pool(name="psPV", bufs=2, space="PSUM") as psPV,
            tc.tile_pool(name="dram", bufs=4, space="DRAM") as dramp,
        ):
            # --- one-time loads -------------------------------------------
            xt_sb = constp.tile([128, NC, S], BF16, tag="xt")
            for c in range(NC):
                nc.sync.dma_start(xt_sb[:, c, :], xT[:, c, :])
            wo_sb = constp.tile([128, NC, SQT], BF16, tag="wo")
            nc.sync.dma_start(wo_sb[:], woT[:])
            cos_sb = constp.tile([128, S], BF16, tag="cos")
            nc.sync.dma_start(cos_sb[:], cosT[:])
            sin_sb = constp.tile([128, S], BF16, tag="sin")
            nc.sync.dma_start(sin_sb[:], sinT[:])
            mask_sb = constp.tile([128, 4, SQT], BF16, tag="mask")
            nc.sync.dma_start(mask_sb[:], masks[:])
            ident = constp.tile([128, 128], BF16, tag="ident")
            make_identity(nc, ident[:])

            out_acc = accp.tile([128, NSB, SQT], F32, tag="oacc")

            def emit_wo_chunk(hp, j, ag0c, ag1c):
                # Wo partial for s rows [512j, 512j+512) of head pair hp.
                final = hp == NHP - 1
                for t in range(SQT // 128):
                    sb = (SQT // 128) * j + t
                    tsl = bass.ts(t, 128)
                    ps = psA.tile([128, SQT], F32, tag="psA")
                    nc.tensor.matmul(ps[:], ag0c[:, tsl], wo_sb[:, hp, :],
                                     start=True, stop=False)
                    nc.tensor.matmul(ps[:], ag1c[:, tsl],
                                     wo_sb[:, NC // 2 + hp, :],
                                     start=False, stop=True)
                    if hp == 0:
                        nc.vector.tensor_copy(out_acc[:, sb, :], ps[:])
                    else:
                        nc.vector.tensor_tensor(
                            out=out_acc[:, sb, :], in0=out_acc[:, sb, :],
                            in1=ps[:], op=add)
                    if final:
                        nc.sync.dma_start(out[bass.ts(sb, 128), :],
                                          out_acc[:, sb, :])

            pending = []
            for hp in range(NHP):
                # --- load this head-pair's weight slices ------------------
                wq_sb = wp.tile([128, NC, 128], BF16, tag="wq")
                nc.sync.dma_start(wq_sb[:], wqT[:, :, hp, :])
                wk_sb = wp.tile([128, NC, 128], BF16, tag="wk")
                nc.sync.dma_start(wk_sb[:], wkT[:, :, hp, :])
                wv_sb = wp.tile([128, NC, 128], BF16, tag="wv")
                nc.sync.dma_start(wv_sb[:], wvT[:, :, hp, :])

                # --- q/k projections + RoPE -------------------------------
                qT2 = qkp.tile([128, S], BF16, tag="qT")
                kT2 = qkp.tile([128, S], BF16, tag="kT")
                for w_sb, dst in ((wq_sb, qT2), (wk_sb, kT2)):
                    for j in range(NSQ):
                        jsl = bass.ts(j, SQT)
                        ps = psA.tile([128, SQT], F32, tag="psA")
                        for c in range(NC):
                            nc.tensor.matmul(
                                ps[:], w_sb[:, c, :], xt_sb[:, c, jsl],
                                start=(c == 0), stop=(c == NC - 1))
                        # evacuate PSUM first (frees the bank after one
                        # pass), then rotate in bf16:
                        # dst = q*cos + shuffle16(q)*sin_signed
                        qsb = smallp.tile([128, SQT], BF16, tag="qsb")
                        nc.vector.tensor_copy(qsb[:], ps[:])
                        t1 = ropep.tile([128, SQT], BF16, tag="t1")
                        nc.vector.tensor_tensor(
                            out=t1[:], in0=qsb[:], in1=cos_sb[:, jsl], op=mult)
                        sh = ropep.tile([128, SQT], BF16, tag="sh")
                        nc.vector.stream_shuffle(sh[:], qsb[:], SWAP16)
                        t2 = ropep.tile([128, SQT], BF16, tag="t2")
                        nc.vector.tensor_tensor(
                            out=t2[:], in0=sh[:], in1=sin_sb[:, jsl], op=mult)
                        nc.vector.tensor_tensor(
                            out=dst[:, jsl], in0=t1[:], in1=t2[:], op=add)

                # --- v projection + transpose + ones column ---------------
                vaug = vp.tile([128, NSB, 130], BF16, tag="vaug")
                nc.gpsimd.memset(vaug[:, :, 64], 1.0)
                nc.gpsimd.memset(vaug[:, :, 129], 1.0)
                for j in range(NSQ):
                    jsl = bass.ts(j, SQT)
                    ps = psA.tile([128, SQT], F32, tag="psA")
                    for c in range(NC):
                        nc.tensor.matmul(
                            ps[:], wv_sb[:, c, :], xt_sb[:, c, jsl],
                            start=(c == 0), stop=(c == NC - 1))
                    vt_sb = smallp.tile([128, SQT], BF16, tag="vt")
                    nc.vector.tensor_copy(vt_sb[:], ps[:])
                    for t in range(SQT // 128):
                        sb = j * (SQT // 128) + t
                        tp = psA.tile([128, 128], BF16, tag="psA")
                        nc.tensor.transpose(
                            tp[:], vt_sb[:, bass.ts(t, 128)], ident[:])
                        nc.vector.tensor_copy(vaug[:, sb, 0:64], tp[:, 0:64])
                        nc.vector.tensor_copy(vaug[:, sb, 65:129], tp[:, 64:128])

                # --- causal attention, scores held transposed -------------
                attnT2 = attnp.tile([128, S], BF16, tag="attnT")
                unnorm = unnp.tile([128, S], F32, tag="unnorm")
                for j in range(NSQ):
                    jsl = bass.ts(j, SQT)
                    pv0 = psPV.tile([128, SQT], F32, tag="pv")
                    pv1 = psPV.tile([128, SQT], F32, tag="pv")
                    n_sk = (SQT // 128) * j + 4
                    for i in range(n_sk):
                        qk2 = psQK.tile([128, 2 * SQT], F32, tag="qk")
                        for h in range(2):
                            nc.tensor.matmul(
                                qk2[:, bass.ts(h, SQT)],
                                kT2[64 * h:64 * h + 64, bass.ts(i, 128)],
                                qT2[64 * h:64 * h + 64, jsl],
                                start=True, stop=True)
                        pr2 = probsp.tile([128, 2 * SQT], BF16, tag="probs")
                        nc.scalar.activation(pr2[:], qk2[:], Exp, scale=0.125)
                        m = i - (SQT // 128) * j
                        if m >= 0:
                            nc.vector.tensor_tensor(
                                out=pr2[:].rearrange("p (two f) -> p two f", two=2),
                                in0=pr2[:].rearrange("p (two f) -> p two f", two=2),
                                in1=mask_sb[:, m, :].unsqueeze(1).broadcast_to(
                                    [128, 2, SQT]),
                                op=mult)
                        for h, pv in ((0, pv0), (1, pv1)):
                            nc.tensor.matmul(
                                pv[0:65, :], vaug[:, i, 65 * h:65 * h + 65],
                                pr2[:, bass.ts(h, SQT)],
                                start=(i == 0), stop=(i == n_sk - 1))

                    # stage PV results + denominators (rows 0/32), free PSUM
                    den = normp.tile([64, SQT], F32, tag="den")
                    nc.gpsimd.memset(den[:], 1.0)
                    for h, pv in ((0, pv0), (1, pv1)):
                        nc.vector.tensor_copy(
                            unnorm[64 * h:64 * h + 64, jsl], pv[0:64, :])
                        nc.vector.tensor_copy(
                            den[32 * h:32 * h + 1, :], pv[64:65, :])

                    # Wo filler for already-gathered earlier slices
                    n_pop = 2 if hp == NHP - 1 and j > 0 else 1
                    for _ in range(n_pop):
                        if pending:
                            emit_wo_chunk(*pending.pop(0))

                    # normalize this slice and ship it
                    rec = normp.tile([64, SQT], F32, tag="recb")
                    nc.vector.reciprocal(rec[:], den[:])
                    for h in range(2):
                        if h == 0:
                            rin = rec[0:1, :]
                        else:
                            r1 = normp.tile([1, SQT], F32, tag="r1")
                            nc.vector.tensor_copy(r1[:], rec[32:33, :])
                            rin = r1[:]
                        rec128 = normp.tile([128, SQT], F32, tag="rec128")
                        nc.gpsimd.partition_broadcast(rec128[:], rin)
                        nc.vector.tensor_tensor(
                            out=attnT2[64 * h:64 * h + 64, jsl],
                            in0=unnorm[64 * h:64 * h + 64, jsl],
                            in1=rec128[64 * h:64 * h + 64, :], op=mult)

                    ag_in = dramp.tile([128, SQT], BF16, tag="ag_in")
                    nc.sync.dma_start(ag_in[:], attnT2[:, jsl])
                    ag_out = dramp.tile([2, 128, SQT], BF16, tag="ag_out")
                    nc.gpsimd.collective_compute(
                        "AllGather", mybir.AluOpType.bypass,
                        ins=[ag_in[:].opt()], outs=[ag_out[:].opt()],
                        replica_groups=groups)
                    ag0c = agp.tile([128, SQT], BF16, tag="ag0c")
                    nc.sync.dma_start(ag0c[:], ag_out[0])
                    ag1c = agp.tile([128, SQT], BF16, tag="ag1c")
                    nc.sync.dma_start(ag1c[:], ag_out[1])
                    pending.append((hp, j, ag0c, ag1c))

            while pending:
                emit_wo_chunk(*pending.pop(0))

    nc.compile()
    return nc


# ---------------------------------------------------------------------------
# Host-side sharding / unsharding
# ---------------------------------------------------------------------------

def _host_inputs(x, Wq, Wk, Wv, Wo, token_positions, n_cores, S):
    import ml_dtypes
    bf16 = ml_dtypes.bfloat16
    D = D_MODEL
    NC = D // 128
    NHP = 4

    # rope tables.  Partition layout within each head (64 partitions):
    # [e0..e15, o0..o15, e16..e31, o16..o31] -- the rotation partner sits
    # 16 partitions away inside the same 32-group, so the kernel's
    # stream_shuffle (a per-32-group lane shuffle) can realize the swap.
    pos = np.asarray(token_positions).astype(np.float32)  # (S,)
    i32 = np.arange(32, dtype=np.float32)
    inv_freq = ROPE_THETA ** (-i32 / 32.0)
    ang = pos[None, :] * inv_freq[:, None]              # (32, S)
    p = np.arange(128)
    pp = p % 64
    g, o = pp // 32, pp % 32
    freq_idx = 16 * g + (o % 16)                        # (128,)
    sign = np.where(o % 32 < 16, -1.0, 1.0)             # even slots: -sin
    cosT = np.cos(ang[freq_idx, :]).astype(bf16)        # (128, S)
    sinT = (sign[:, None] * np.sin(ang[freq_idx, :])).astype(bf16)

    # causal mask patterns for the 4 diagonal block offsets
    pcol = np.arange(128)[:, None]
    f = np.arange(SQT)[None, :]
    masks = np.stack([(pcol + 128 * m <= f) for m in range(4)], axis=1)
    masks = masks.astype(bf16)                          # (128, 4, 512)

    # de-interleaving row permutation for q/k (see rope table comment)
    def qk_rows(grp):
        rows = []
        for h in range(8 * grp, 8 * grp + 8):
            rows += [h * DH + 2 * i for i in range(16)]
            rows += [h * DH + 2 * i + 1 for i in range(16)]
            rows += [h * DH + 2 * i for i in range(16, 32)]
            rows += [h * DH + 2 * i + 1 for i in range(16, 32)]
        return rows

    def wqk_layout(W, grp):
        # (D, 512) -> [128, NC, NHP, 128]
        t = W[qk_rows(grp), :].T.astype(bf16)
        return np.ascontiguousarray(
            t.reshape(NC, 128, NHP, 128).transpose(1, 0, 2, 3))

    def wv_layout(W, grp):
        t = W[512 * grp:512 * grp + 512, :].T.astype(bf16)
        return np.ascontiguousarray(
            t.reshape(NC, 128, NHP, 128).transpose(1, 0, 2, 3))

    def wo_layout(W, grp):
        t = W.T[:, 512 * grp:512 * grp + 512].astype(bf16)  # (D, 512)
        return np.ascontiguousarray(t.reshape(NC, 128, SQT).transpose(1, 0, 2))

    in_maps = []
    for c in range(n_cores):
        b, grp = c // 2, c % 2
        xb = np.ascontiguousarray(x[b].T).astype(bf16)  # (D, S)
        in_maps.append({
            "xT": np.ascontiguousarray(
                xb.reshape(NC, 128, S).transpose(1, 0, 2)),
            "wqT": wqk_layout(Wq, grp),
            "wkT": wqk_layout(Wk, grp),
            "wvT": wv_layout(Wv, grp),
            "woT": wo_layout(Wo, grp),
            "cosT": cosT,
            "sinT": sinT,
            "masks": masks,
        })
    return in_maps


def _assemble(results, n_cores, S):
    B = n_cores // 2
    full = np.empty((B, S, D_MODEL), dtype=np.float32)
    for c in range(n_cores):
        b, grp = c // 2, c % 2
        full[b, :, 512 * grp:512 * grp + 512] = results[c]["out"]
    return full


# ---------------------------------------------------------------------------
# Entry point
# ---------------------------------------------------------------------------

_NC_CACHE = {}


def _get_nc(n_cores, S):
    key = (n_cores, S)
    if key not in _NC_CACHE:
        _NC_CACHE[key] = build_kernel(n_cores, S)
    return _NC_CACHE[key]


def kernel(x, Wq, Wk, Wv, Wo, token_positions, _trace=False, _tmpdir=None):
    from concourse.bass_utils import run_bass_kernel_spmd

    x = np.asarray(x)
    B, S, D = x.shape
    n_cores = 2 * B
    nc = _get_nc(n_cores, S)
    in_maps = _host_inputs(np.asarray(x), np.asarray(Wq), np.asarray(Wk),
                           np.asarray(Wv), np.asarray(Wo),
                           np.asarray(token_positions), n_cores, S)
    res = run_bass_kernel_spmd(nc, in_maps, core_ids=list(range(n_cores)),
                               trace=_trace, tmpdir=_tmpdir)
    out = _assemble(res.results, n_cores, S)
    if _trace:
        return out, res
    return out
